# revision 1
# baseline (speedup 1.0000x reference)
"""Trainium2 Bass kernel for nn_Contextual_MFN (Memory Fusion Network).

Structure (per core; batch data-parallel 8 ways, 32 rows/core):
  phase 0: xWb[t] = Wih_aug @ x_aug[t]  (all t, fp32r matmuls, bias folded in)
  phase 1: sequential 3xLSTM recurrence; gates = xWb (identity-inject) + Whh@h
  phase 2a: time-parallel attention: att1 MLP -> exp -> U = E*cStar (unnormalized),
            S = sum(E), att2/g1/g2 linear parts on U, bias*S folds
  recip:   Sinv = 1/S
  phase 3: sequential memory-gate recurrence (mem-dependent matmuls only)
  phase 4: output MLP on [h_l, h_a, h_v, mem]

All activations feature-major: [features(partitions), batch(free)].
"""
import os
import numpy as np

import concourse.bass as bass
import concourse.tile as tile
from concourse import bacc, mybir
from concourse.bass_utils import run_bass_kernel_spmd

F32 = mybir.dt.float32
USE_F32R = True
F32R = mybir.dt.float32r
AF = mybir.ActivationFunctionType

# Problem constants (hardcoded; kernel.py must be self-contained)
T_FULL = 512
NBATCH = 256
NCORES = 8
B = NBATCH // NCORES          # 32 batch rows per core
D_L, D_A, D_V = 300, 74, 35
DIN = D_L + D_A + D_V         # 409
DAUG = DIN + 1                # 410 (ones row for bias)
DH = 128
MEM = 256
CH0 = 16                      # phase-0 chunk (steps)
CH2 = 8                       # phase-2a / phase-3 chunk (steps)

# gate slot order: s = g'*3 + m, with g' in (i, f, o, g_tanh); torch rows are (i, f, g, o)
TORCH_G = (0, 1, 3, 2)        # our slot g' -> torch gate row block


def _nonzero_kcs(s):
    """Phase-0 K-chunks (of Waug rows 0..409 padded to 512) that are nonzero for
    output slot s. m=0 (l): feats 0-299 -> kc 0,1,2 (+ones kc3). m=1 (a): 300-373
    -> kc2 (+kc3 ones). m=2 (v): 374-408 -> kc2,kc3 (+ones kc3)."""
    m = s % 3
    if m == 0:
        return [0, 1, 2, 3]
    return [2, 3]


def build_program(Tp=T_FULL):
    global F32R
    F32R = mybir.dt.float32r if USE_F32R else F32
    assert Tp % CH0 == 0 and Tp % CH2 == 0
    NCH2 = Tp // CH2
    nc = bacc.Bacc("TRN2", target_bir_lowering=False, debug=False)

    # ---------------- external inputs ----------------
    xT = nc.dram_tensor("xT", [DAUG, Tp * B], F32, kind="ExternalInput")
    waug = nc.dram_tensor("waug", [512, 1536], F32, kind="ExternalInput")
    whhT = nc.dram_tensor("whhT", [128, 1536], F32, kind="ExternalInput")
    ident = nc.dram_tensor("ident", [128, 128], F32, kind="ExternalInput")
    ones128 = nc.dram_tensor("ones128", [128, 1], F32, kind="ExternalInput")

    a1w1 = nc.dram_tensor("a1w1", [768, 256], F32, kind="ExternalInput")
    a1b1 = nc.dram_tensor("a1b1", [128, 2], F32, kind="ExternalInput")
    a1w2 = nc.dram_tensor("a1w2", [256, 768], F32, kind="ExternalInput")
    a1b2 = nc.dram_tensor("a1b2", [128, 6], F32, kind="ExternalInput")
    a2w1 = nc.dram_tensor("a2w1", [768, 256], F32, kind="ExternalInput")
    a2b1r = nc.dram_tensor("a2b1r", [1, 256], F32, kind="ExternalInput")
    a2w2 = nc.dram_tensor("a2w2", [256, 256], F32, kind="ExternalInput")
    a2b2r = nc.dram_tensor("a2b2r", [1, 256], F32, kind="ExternalInput")
    g1a = nc.dram_tensor("g1a", [768, 256], F32, kind="ExternalInput")
    g2a = nc.dram_tensor("g2a", [768, 256], F32, kind="ExternalInput")
    g1b = nc.dram_tensor("g1b", [256, 256], F32, kind="ExternalInput")
    g2b = nc.dram_tensor("g2b", [256, 256], F32, kind="ExternalInput")
    g1b1r = nc.dram_tensor("g1b1r", [1, 256], F32, kind="ExternalInput")
    g2b1r = nc.dram_tensor("g2b1r", [1, 256], F32, kind="ExternalInput")
    g1w2 = nc.dram_tensor("g1w2", [256, 256], F32, kind="ExternalInput")
    g2w2 = nc.dram_tensor("g2w2", [256, 256], F32, kind="ExternalInput")
    gb2r = nc.dram_tensor("gb2r", [1, 512], F32, kind="ExternalInput")
    ow1 = nc.dram_tensor("ow1", [640, 256], F32, kind="ExternalInput")
    ob1 = nc.dram_tensor("ob1", [128, 2], F32, kind="ExternalInput")
    ow2 = nc.dram_tensor("ow2", [256, 1], F32, kind="ExternalInput")
    ob2 = nc.dram_tensor("ob2", [1, 1], F32, kind="ExternalInput")

    out_d = nc.dram_tensor("out", [B, 1], F32, kind="ExternalOutput")

    # ---------------- internal dram scratch ----------------
    xwb = nc.dram_tensor("xwb", [12, Tp, 128, B], F32)
    cs = [nc.dram_tensor(f"cseq{m}", [Tp + 1, 128, B], F32) for m in range(3)]
    a2r_d = nc.dram_tensor("a2r_d", [NCH2, 128, 2 * CH2 * B], F32)
    g1p_d = nc.dram_tensor("g1p_d", [NCH2, 128, 2 * CH2 * B], F32)
    g2p_d = nc.dram_tensor("g2p_d", [NCH2, 128, 2 * CH2 * B], F32)
    s_d = nc.dram_tensor("s_d", [NCH2, CH2 * B], F32)
    sinv_d = nc.dram_tensor("sinv_d", [NCH2, CH2 * B], F32)

    NB2 = CH2 * B  # 256: phase-2a matmul free dim

    import contextlib
    with tile.TileContext(nc) as tc:
        ctx = contextlib.ExitStack()
        with ctx:
            wpool = ctx.enter_context(tc.tile_pool(name="weights", bufs=1))
            hpool = ctx.enter_context(tc.tile_pool(name="hstate", bufs=2))

            # ---- resident weights / constants in SBUF ----
            wihT_t = wpool.tile([128, 4, 1536], F32R)
            nc.sync.dma_start(
                wihT_t[:], waug.ap().rearrange("(kc p) c -> p kc c", p=128).bitcast(F32R))
            whhT_t = wpool.tile([128, 1536], F32)
            nc.sync.dma_start(whhT_t[:], whhT.ap())
            id_t = wpool.tile([128, 128], F32R)
            nc.sync.dma_start(id_t[:], ident.ap().bitcast(F32R))
            ones128_t = wpool.tile([128, 1], F32R)
            nc.sync.dma_start(ones128_t[:], ones128.ap().bitcast(F32R))

            a1w1_t = wpool.tile([128, 6, 256], F32R)
            nc.sync.dma_start(a1w1_t[:], a1w1.ap().rearrange("(kc p) c -> p kc c", p=128).bitcast(F32R))
            a1b1_t = wpool.tile([128, 2], F32)
            nc.sync.dma_start(a1b1_t[:], a1b1.ap())
            a1w2_t = wpool.tile([128, 2, 768], F32R)
            nc.sync.dma_start(a1w2_t[:], a1w2.ap().rearrange("(kc p) c -> p kc c", p=128).bitcast(F32R))
            a1b2_t = wpool.tile([128, 6], F32)
            nc.sync.dma_start(a1b2_t[:], a1b2.ap())
            a2w1_t = wpool.tile([128, 6, 256], F32R)
            nc.sync.dma_start(a2w1_t[:], a2w1.ap().rearrange("(kc p) c -> p kc c", p=128).bitcast(F32R))
            a2b1r_t = wpool.tile([1, 256], F32R)
            nc.sync.dma_start(a2b1r_t[:], a2b1r.ap().bitcast(F32R))
            a2w2_t = wpool.tile([128, 2, 256], F32R)
            nc.sync.dma_start(a2w2_t[:], a2w2.ap().rearrange("(kc p) c -> p kc c", p=128).bitcast(F32R))
            a2b2r_t = wpool.tile([1, 256], F32R)
            nc.sync.dma_start(a2b2r_t[:], a2b2r.ap().bitcast(F32R))
            g1a_t = wpool.tile([128, 6, 256], F32R)
            nc.sync.dma_start(g1a_t[:], g1a.ap().rearrange("(kc p) c -> p kc c", p=128).bitcast(F32R))
            g2a_t = wpool.tile([128, 6, 256], F32R)
            nc.sync.dma_start(g2a_t[:], g2a.ap().rearrange("(kc p) c -> p kc c", p=128).bitcast(F32R))
            g1b1r_t = wpool.tile([1, 256], F32R)
            nc.sync.dma_start(g1b1r_t[:], g1b1r.ap().bitcast(F32R))
            g2b1r_t = wpool.tile([1, 256], F32R)
            nc.sync.dma_start(g2b1r_t[:], g2b1r.ap().bitcast(F32R))
            g1b_t = wpool.tile([128, 2, 256], F32)
            nc.sync.dma_start(g1b_t[:], g1b.ap().rearrange("(kc p) c -> p kc c", p=128))
            g2b_t = wpool.tile([128, 2, 256], F32)
            nc.sync.dma_start(g2b_t[:], g2b.ap().rearrange("(kc p) c -> p kc c", p=128))
            g1w2_t = wpool.tile([128, 2, 256], F32)
            nc.sync.dma_start(g1w2_t[:], g1w2.ap().rearrange("(kc p) c -> p kc c", p=128))
            g2w2_t = wpool.tile([128, 2, 256], F32)
            nc.sync.dma_start(g2w2_t[:], g2w2.ap().rearrange("(kc p) c -> p kc c", p=128))
            gb2r_t = wpool.tile([1, 512], F32)
            nc.sync.dma_start(gb2r_t[:], gb2r.ap())
            ow1_t = wpool.tile([128, 5, 256], F32)
            nc.sync.dma_start(ow1_t[:], ow1.ap().rearrange("(kc p) c -> p kc c", p=128))
            ob1_t = wpool.tile([128, 2], F32)
            nc.sync.dma_start(ob1_t[:], ob1.ap())
            ow2_t = wpool.tile([128, 2, 1], F32)
            nc.sync.dma_start(ow2_t[:], ow2.ap().rearrange("(kc p) c -> p kc c", p=128))
            ob2_t = wpool.tile([1, 1], F32)
            nc.sync.dma_start(ob2_t[:], ob2.ap())

            ones32_t = wpool.tile([1, 32], F32)
            nc.vector.memset(ones32_t[:], 1.0)
            ones1x128_t = wpool.tile([1, 128], F32)
            nc.vector.memset(ones1x128_t[:], 1.0)
            zero_t = wpool.tile([128, 32], F32)
            nc.vector.memset(zero_t[:], 0.0)

            # =================== PHASE 0: xWb ===================
            with (
                tc.tile_pool(name="p0x", bufs=2) as p0x,
                tc.tile_pool(name="p0s", bufs=4) as p0s,
                tc.tile_pool(name="p0p", bufs=4, space="PSUM") as p0p,
            ):
                for k0 in range(Tp // CH0):
                    t0 = k0 * CH0
                    n0 = CH0 * B  # 512
                    xt = p0x.tile([128, 4, n0], F32R, tag="xt")
                    for kc in range(4):
                        rows = 128 if kc < 3 else DAUG - 384  # 26 on last chunk
                        nc.sync.dma_start(
                            xt[0:rows, kc, :],
                            xT.ap()[kc * 128:kc * 128 + rows, t0 * B:(t0 + CH0) * B].bitcast(F32R))
                    for s in range(12):
                        pt = p0p.tile([128, n0], F32, tag="p0acc")
                        kcs = _nonzero_kcs(s)
                        for i, kc in enumerate(kcs):
                            rows = 128 if kc < 3 else DAUG - 384
                            nc.tensor.matmul(
                                pt[:], wihT_t[0:rows, kc, s * 128:(s + 1) * 128],
                                xt[0:rows, kc, :],
                                start=(i == 0), stop=(i == len(kcs) - 1))
                        st = p0s.tile([128, n0], F32, tag="p0st")
                        if s % 2 == 0:
                            nc.vector.tensor_copy(st[:], pt[:])
                        else:
                            nc.scalar.copy(st[:], pt[:])
                        # dram [CH0, 128, B] slab, partition-major write
                        nc.sync.dma_start(
                            xwb.ap()[s, t0:t0 + CH0, :, :].transpose([1, 0, 2]),
                            st[:].rearrange("p (t b) -> p t b", b=B))

            tc.strict_bb_all_engine_barrier()

            # =================== PHASE 1: LSTM recurrence ===================
            with (
                tc.tile_pool(name="p1w", bufs=2) as p1w,
                tc.tile_pool(name="p1s", bufs=3) as p1s,
                tc.tile_pool(name="p1c", bufs=4) as p1c,
                tc.tile_pool(name="p1p", bufs=2, space="PSUM") as p1p,
            ):
                h_cur = hpool.tile([128, 96], F32, tag="h")
                nc.vector.memset(h_cur[:], 0.0)
                c_cur = p1c.tile([128, 3, 32], F32, tag="c")
                nc.vector.memset(c_cur[:], 0.0)
                for m in range(3):
                    nc.sync.dma_start(cs[m].ap()[0], zero_t[:])

                h_fin = None
                for w in range(Tp // CH2):
                    t0 = w * CH2
                    win = p1w.tile([128, 12, CH2, 32], F32R, tag="xwbwin")
                    for s in range(12):
                        nc.sync.dma_start(
                            win[:, s, :, :],
                            xwb.ap()[s, t0:t0 + CH2, :, :].transpose([1, 0, 2]).bitcast(F32R))
                    for j in range(CH2):
                        t = t0 + j
                        gp = p1p.tile([128, 12, 32], F32, tag="gates")
                        nc.tensor.matmul(gp[:], id_t[:], win[:, :, j, :],
                                         start=True, stop=False)
                        for s in range(12):
                            gq, m = divmod(s, 3)
                            nc.tensor.matmul(
                                gp[:, s, :],
                                whhT_t[:, s * 128:(s + 1) * 128],
                                h_cur[:, m * 32:(m + 1) * 32],
                                start=False, stop=(s == 11))
                        sg = p1s.tile([128, 9, 32], F32, tag="sg")
                        nc.scalar.activation(sg[:], gp[:, 0:9, :], AF.Sigmoid)
                        tg = p1s.tile([128, 3, 32], F32, tag="tg")
                        nc.scalar.activation(tg[:], gp[:, 9:12, :], AF.Tanh)
                        t1 = p1s.tile([128, 3, 32], F32, tag="t1")
                        nc.vector.tensor_mul(t1[:], sg[:, 0:3, :], tg[:])
                        t2 = p1s.tile([128, 3, 32], F32, tag="t2")
                        nc.vector.tensor_mul(t2[:], sg[:, 3:6, :], c_cur[:])
                        c_new = p1c.tile([128, 3, 32], F32, tag="c")
                        nc.vector.tensor_add(c_new[:], t1[:], t2[:])
                        tc_t = p1s.tile([128, 3, 32], F32, tag="tc")
                        nc.scalar.activation(tc_t[:], c_new[:], AF.Tanh)
                        h_new = hpool.tile([128, 96], F32, tag="h")
                        nc.vector.tensor_mul(
                            h_new[:].rearrange("p (m b) -> p m b", b=32),
                            sg[:, 6:9, :], tc_t[:])
                        for m in range(3):
                            nc.sync.dma_start(cs[m].ap()[t + 1], c_new[:, m, :])
                        c_cur = c_new
                        h_cur = h_new
                h_fin = h_cur

            tc.strict_bb_all_engine_barrier()

            # =================== PHASE 2a: time-parallel attention ===================
            with (
                tc.tile_pool(name="p2c", bufs=2) as p2c,
                tc.tile_pool(name="p2s", bufs=2) as p2s,
                tc.tile_pool(name="p2r", bufs=3) as p2r,
                tc.tile_pool(name="p2p1", bufs=2, space="PSUM") as p2p1,
                tc.tile_pool(name="p2pe", bufs=1, space="PSUM") as p2pe,
                tc.tile_pool(name="p2po", bufs=2, space="PSUM") as p2po,
                tc.tile_pool(name="p2ps", bufs=1, space="PSUM") as p2ps,
            ):
                for k in range(NCH2):
                    t0 = k * CH2
                    cw = [p2c.tile([128, CH2 + 1, 32], F32R, tag=f"cw{m}", name=f"cw{m}")
                          for m in range(3)]
                    for m in range(3):
                        nc.sync.dma_start(
                            cw[m][:], cs[m].ap()[t0:t0 + CH2 + 1].transpose([1, 0, 2]).bitcast(F32R))

                    def rhs_k(kc):
                        if kc < 3:
                            return cw[kc][:, 0:CH2, :]
                        return cw[kc - 3][:, 1:CH2 + 1, :]

                    # att1 layer 1 + relu
                    y1p = p2p1.tile([128, 2, NB2], F32, tag="stage1")
                    for mc in range(2):
                        for kc in range(6):
                            nc.tensor.matmul(
                                y1p[:, mc, :], a1w1_t[:, kc, mc * 128:(mc + 1) * 128],
                                rhs_k(kc), start=(kc == 0), stop=(kc == 5))
                    y1 = p2s.tile([128, 2, NB2], F32R, tag="y1")
                    for mc in range(2):
                        nc.scalar.activation(y1[:, mc, :], y1p[:, mc, :], AF.Relu,
                                             bias=a1b1_t[:, mc:mc + 1])
                    # att1 layer 2 + exp
                    ep = p2pe.tile([128, 6, NB2], F32, tag="logits")
                    for mc6 in range(6):
                        for kc in range(2):
                            nc.tensor.matmul(
                                ep[:, mc6, :], a1w2_t[:, kc, mc6 * 128:(mc6 + 1) * 128],
                                y1[:, kc, :], start=(kc == 0), stop=(kc == 1))
                    et = p2s.tile([128, 6, NB2], F32R, tag="et")
                    for mc6 in range(6):
                        nc.scalar.activation(et[:, mc6, :], ep[:, mc6, :], AF.Exp,
                                             bias=a1b2_t[:, mc6:mc6 + 1])
                    # U = E * cStar (unnormalized attended)
                    ut = p2s.tile([128, 6, NB2], F32R, tag="ut")
                    for q in range(6):
                        nc.vector.tensor_mul(ut[:, q, :], et[:, q, :].bitcast(F32),
                                             rhs_k(q).bitcast(F32))
                    # S = sum over features of E
                    sp = p2ps.tile([1, NB2], F32, tag="srow")
                    for q in range(6):
                        nc.tensor.matmul(sp[:], ones128_t[:], et[:, q, :],
                                         start=(q == 0), stop=(q == 5))
                    srow = p2r.tile([1, NB2], F32R, tag="srow_s")
                    nc.vector.tensor_copy(srow[:], sp[:])
                    nc.sync.dma_start(s_d.ap()[k:k + 1, :], srow[:].bitcast(F32))

                    # att2 layer 1 + relu
                    zp = p2p1.tile([128, 2, NB2], F32, tag="stage1")
                    for mc in range(2):
                        for kc in range(6):
                            nc.tensor.matmul(
                                zp[:, mc, :], a2w1_t[:, kc, mc * 128:(mc + 1) * 128],
                                ut[:, kc, :], start=(kc == 0), stop=False)
                        nc.tensor.matmul(zp[:, mc, :], a2b1r_t[:, mc * 128:(mc + 1) * 128],
                                         srow[:], start=False, stop=True)
                    z = p2s.tile([128, 2, NB2], F32R, tag="z")
                    for mc in range(2):
                        nc.scalar.activation(z[:, mc, :], zp[:, mc, :], AF.Relu)
                    # att2 layer 2 (raw) + b2*S fold
                    ap2 = p2po.tile([128, 2, NB2], F32, tag="out")
                    for mc in range(2):
                        for kc in range(2):
                            nc.tensor.matmul(
                                ap2[:, mc, :], a2w2_t[:, kc, mc * 128:(mc + 1) * 128],
                                z[:, kc, :], start=(kc == 0), stop=False)
                        nc.tensor.matmul(ap2[:, mc, :], a2b2r_t[:, mc * 128:(mc + 1) * 128],
                                         srow[:], start=False, stop=True)
                    a2s = p2s.tile([128, 2, NB2], F32, tag="a2s")
                    nc.scalar.copy(a2s[:], ap2[:])
                    nc.sync.dma_start(a2r_d.ap()[k], a2s[:].rearrange("p a b -> p (a b)"))

                    # g1 / g2 attended-part + b1*S fold
                    for gi, (gw, gbr, gd) in enumerate(
                            ((g1a_t, g1b1r_t, g1p_d), (g2a_t, g2b1r_t, g2p_d))):
                        gp2 = p2po.tile([128, 2, NB2], F32, tag="out")
                        for mc in range(2):
                            for kc in range(6):
                                nc.tensor.matmul(
                                    gp2[:, mc, :], gw[:, kc, mc * 128:(mc + 1) * 128],
                                    ut[:, kc, :], start=(kc == 0), stop=False)
                            nc.tensor.matmul(gp2[:, mc, :], gbr[:, mc * 128:(mc + 1) * 128],
                                             srow[:], start=False, stop=True)
                        gs = p2s.tile([128, 2, NB2], F32, tag=f"g{gi}s")
                        if gi == 0:
                            nc.vector.tensor_copy(gs[:], gp2[:])
                        else:
                            nc.scalar.copy(gs[:], gp2[:])
                        nc.sync.dma_start(gd.ap()[k], gs[:].rearrange("p a b -> p (a b)"))

            tc.strict_bb_all_engine_barrier()

            # =================== reciprocal of S ===================
            with tc.tile_pool(name="prc", bufs=1) as prc:
                nrows = NCH2
                sall = prc.tile([nrows, NB2], F32)
                nc.sync.dma_start(sall[:], s_d.ap())
                sinv = prc.tile([nrows, NB2], F32)
                nc.vector.reciprocal(sinv[:], sall[:])
                nc.sync.dma_start(sinv_d.ap(), sinv[:])

            tc.strict_bb_all_engine_barrier()

            # =================== PHASE 3: memory recurrence ===================
            with (
                tc.tile_pool(name="p3w", bufs=2) as p3w,
                tc.tile_pool(name="p3s", bufs=3) as p3s,
                tc.tile_pool(name="p3m", bufs=2) as p3m,
                tc.tile_pool(name="p3p", bufs=2, space="PSUM") as p3p,
                tc.tile_pool(name="p3pb", bufs=2, space="PSUM") as p3pb,
            ):
                mem_cur = p3m.tile([128, 2, 32], F32, tag="mem")
                nc.vector.memset(mem_cur[:], 0.0)
                for k in range(NCH2):
                    aw = p3w.tile([128, 2, CH2, 32], F32, tag="aw")
                    nc.sync.dma_start(aw[:], a2r_d.ap()[k].rearrange("p (a t b) -> p a t b", a=2, b=32))
                    g1w_ = p3w.tile([128, 2, CH2, 32], F32, tag="g1w")
                    nc.sync.dma_start(g1w_[:], g1p_d.ap()[k].rearrange("p (a t b) -> p a t b", a=2, b=32))
                    g2w_ = p3w.tile([128, 2, CH2, 32], F32, tag="g2w")
                    nc.sync.dma_start(g2w_[:], g2p_d.ap()[k].rearrange("p (a t b) -> p a t b", a=2, b=32))
                    sr = p3w.tile([1, NB2], F32, tag="sr")
                    nc.sync.dma_start(sr[:], sinv_d.ap()[k:k + 1, :])

                    for j in range(CH2):
                        # broadcast Sinv_t across partitions via K=1 matmul
                        sb = p3pb.tile([128, 32], F32, tag="sinvb")
                        nc.tensor.matmul(sb[:], ones1x128_t[:], sr[:, j * 32:(j + 1) * 32],
                                         start=True, stop=True)
                        sb2 = sb[:].unsqueeze(1).broadcast_to([128, 2, 32])
                        # normalized g-pre parts
                        u = p3s.tile([128, 4, 32], F32, tag="u")
                        nc.vector.tensor_mul(u[:, 0:2, :], g1w_[:, :, j, :], sb2)
                        nc.vector.tensor_mul(u[:, 2:4, :], g2w_[:, :, j, :], sb2)
                        # mem-part matmuls (+ nothing else: b1*S already folded)
                        pg = p3p.tile([128, 4, 32], F32, tag="gmm")
                        for r, (gwt,) in enumerate(((g1b_t,), (g1b_t,), (g2b_t,), (g2b_t,))):
                            mc = r % 2
                            for kc in range(2):
                                nc.tensor.matmul(
                                    pg[:, r, :], gwt[:, kc, mc * 128:(mc + 1) * 128],
                                    mem_cur[:, kc, :], start=(kc == 0), stop=(kc == 1))
                        w_t = p3s.tile([128, 4, 32], F32, tag="w")
                        nc.vector.tensor_add(w_t[:], u[:], pg[:])
                        hh = p3s.tile([128, 4, 32], F32, tag="hh")
                        nc.scalar.activation(hh[:], w_t[:], AF.Relu)
                        # L2 + b2 fold
                        qg = p3p.tile([128, 4, 32], F32, tag="qmm")
                        for r, gwt in enumerate((g1w2_t, g1w2_t, g2w2_t, g2w2_t)):
                            mc = r % 2
                            goff = 0 if r < 2 else 2
                            for kc in range(2):
                                nc.tensor.matmul(
                                    qg[:, r, :], gwt[:, kc, mc * 128:(mc + 1) * 128],
                                    hh[:, goff + kc, :], start=(kc == 0), stop=False)
                            nc.tensor.matmul(qg[:, r, :], gb2r_t[:, r * 128:(r + 1) * 128],
                                             ones32_t[:], start=False, stop=True)
                        gam = p3s.tile([128, 4, 32], F32, tag="gam")
                        nc.scalar.activation(gam[:], qg[:], AF.Sigmoid)
                        # cHat = tanh(att2raw * Sinv)  (b2*S folded already)
                        v = p3s.tile([128, 2, 32], F32, tag="v")
                        nc.vector.tensor_mul(v[:], aw[:, :, j, :], sb2)
                        ch = p3s.tile([128, 2, 32], F32, tag="ch")
                        nc.scalar.activation(ch[:], v[:], AF.Tanh)
                        # mem = gam1*mem + gam2*cHat
                        m1 = p3s.tile([128, 2, 32], F32, tag="m1")
                        nc.vector.tensor_mul(m1[:], gam[:, 0:2, :], mem_cur[:])
                        m2 = p3s.tile([128, 2, 32], F32, tag="m2")
                        nc.vector.tensor_mul(m2[:], gam[:, 2:4, :], ch[:])
                        mem_new = p3m.tile([128, 2, 32], F32, tag="mem")
                        nc.vector.tensor_add(mem_new[:], m1[:], m2[:])
                        mem_cur = mem_new

                # =================== PHASE 4: output MLP ===================
                with tc.tile_pool(name="p4p", bufs=1, space="PSUM") as p4p:
                    o1p = p4p.tile([128, 2, 32], F32, tag="o1")
                    rhs5 = [h_fin[:, 0:32], h_fin[:, 32:64], h_fin[:, 64:96],
                            mem_cur[:, 0, :], mem_cur[:, 1, :]]
                    for mc in range(2):
                        for kc in range(5):
                            nc.tensor.matmul(
                                o1p[:, mc, :], ow1_t[:, kc, mc * 128:(mc + 1) * 128],
                                rhs5[kc], start=(kc == 0), stop=(kc == 4))
                    o1s = p3s.tile([128, 2, 32], F32, tag="o1s")
                    for mc in range(2):
                        nc.scalar.activation(o1s[:, mc, :], o1p[:, mc, :], AF.Relu,
                                             bias=ob1_t[:, mc:mc + 1])
                    o2p = p4p.tile([1, 32], F32, tag="o2")
                    for kc in range(2):
                        nc.tensor.matmul(o2p[:], ow2_t[:, kc, :], o1s[:, kc, :],
                                         start=(kc == 0), stop=(kc == 1))
                    o2s = p3s.tile([1, 32], F32, tag="o2s")
                    nc.scalar.activation(o2s[:], o2p[:], AF.Identity, bias=ob2_t[:])
                    nc.sync.dma_start(out_d.ap().rearrange("b one -> (one) (b)"), o2s[:])

    nc.compile()
    return nc


# ---------------------------------------------------------------------------
# host-side packing
# ---------------------------------------------------------------------------

def pack_shared(inp):
    """Pack weight tensors (identical across cores)."""
    f = np.float32
    d = {}
    wih = {0: inp["Wih_l"], 1: inp["Wih_a"], 2: inp["Wih_v"]}
    whh = {0: inp["Whh_l"], 1: inp["Whh_a"], 2: inp["Whh_v"]}
    bb = {m: (inp[f"bih_{k}"] + inp[f"bhh_{k}"]).astype(f)
          for m, k in ((0, "l"), (1, "a"), (2, "v"))}
    foff = {0: 0, 1: D_L, 2: D_L + D_A}
    din = {0: D_L, 1: D_A, 2: D_V}

    waug = np.zeros((512, 1536), f)
    whhT = np.zeros((128, 1536), f)
    for gq in range(4):
        tg = TORCH_G[gq]
        for m in range(3):
            s = gq * 3 + m
            wblk = wih[m][tg * 128:(tg + 1) * 128, :]          # [128, din]
            waug[foff[m]:foff[m] + din[m], s * 128:(s + 1) * 128] = wblk.T
            waug[DIN, s * 128:(s + 1) * 128] = bb[m][tg * 128:(tg + 1) * 128]
            whhT[:, s * 128:(s + 1) * 128] = whh[m][tg * 128:(tg + 1) * 128, :].T
    d["waug"] = waug
    d["whhT"] = whhT
    d["ident"] = np.eye(128, dtype=f)
    d["ones128"] = np.ones((128, 1), f)

    d["a1w1"] = inp["att1_W1"].T.astype(f).copy()              # [768, 256]
    d["a1b1"] = inp["att1_b1"].reshape(2, 128).T.astype(f).copy()
    d["a1w2"] = inp["att1_W2"].T.astype(f).copy()              # [256, 768]
    d["a1b2"] = inp["att1_b2"].reshape(6, 128).T.astype(f).copy()
    d["a2w1"] = inp["att2_W1"].T.astype(f).copy()
    d["a2b1r"] = inp["att2_b1"].reshape(1, 256).astype(f).copy()
    d["a2w2"] = inp["att2_W2"].T.astype(f).copy()              # [256, 256]
    d["a2b2r"] = inp["att2_b2"].reshape(1, 256).astype(f).copy()
    d["g1a"] = inp["g1_W1"][:, :768].T.astype(f).copy()
    d["g2a"] = inp["g2_W1"][:, :768].T.astype(f).copy()
    d["g1b"] = inp["g1_W1"][:, 768:].T.astype(f).copy()
    d["g2b"] = inp["g2_W1"][:, 768:].T.astype(f).copy()
    d["g1b1r"] = inp["g1_b1"].reshape(1, 256).astype(f).copy()
    d["g2b1r"] = inp["g2_b1"].reshape(1, 256).astype(f).copy()
    d["g1w2"] = inp["g1_W2"].T.astype(f).copy()
    d["g2w2"] = inp["g2_W2"].T.astype(f).copy()
    d["gb2r"] = np.concatenate([inp["g1_b2"], inp["g2_b2"]]).reshape(1, 512).astype(f)
    d["ow1"] = inp["out_W1"].T.astype(f).copy()                # [640, 256]
    d["ob1"] = inp["out_b1"].reshape(2, 128).T.astype(f).copy()
    d["ow2"] = inp["out_W2"].T.astype(f).copy()                # [256, 1]
    d["ob2"] = inp["out_b2"].reshape(1, 1).astype(f).copy()
    return d


def pack_x(x, core, Tp):
    """x: [Tp, 256, 409] -> xT [410, Tp*B] for one core."""
    xc = np.asarray(x[:, core * B:(core + 1) * B, :], np.float32)   # [Tp, B, 409]
    xt = xc.transpose(2, 0, 1).reshape(DIN, Tp * B)
    return np.concatenate([xt, np.ones((1, Tp * B), np.float32)], 0)


_CACHE = {}


def _get_program(Tp):
    if Tp not in _CACHE:
        _CACHE[Tp] = build_program(Tp)
    return _CACHE[Tp]


def kernel(**inputs):
    x = np.asarray(inputs["x"])
    Tp = x.shape[0]
    nc = _get_program(Tp)
    shared = pack_shared({k: np.asarray(v) for k, v in inputs.items()})
    in_maps = []
    for c in range(NCORES):
        m = dict(shared)
        m["xT"] = np.ascontiguousarray(pack_x(x, c, Tp))
        in_maps.append(m)
    res = run_bass_kernel_spmd(nc, in_maps, list(range(NCORES))).results
    out = np.concatenate([r["out"] for r in res], axis=0)
    return out.astype(np.float32)


if __name__ == "__main__":
    import time
    t0 = time.time()
    nc = build_program(32)
    print("built in", time.time() - t0, "s")



# revision 13
# speedup vs baseline: 2.0065x; 2.0065x over previous
"""Trainium2 Bass kernel for nn_Contextual_MFN (Memory Fusion Network).

Fully-fused single-TileContext design (v2). Batch data-parallel over 8
cores (32 rows/core). Per chunk of CH=8 timesteps, four pipeline stages
flow through SBUF rings with no DRAM round-trips; the tile scheduler
overlaps them across chunks:

  ph0(w): xwb = Wih @ x chunk (time-parallel matmuls, bias via ACT copy)
  ph1(w): 3xLSTM recurrence; gates = inject(xwb) + Whh @ h per step
  ph2a(w): time-parallel attention on the c-sequence; produces
           normalized gate pre-activations u (attended part + b1),
           cHat = tanh(att2 MLP) -- everything not mem-dependent
  ph3(w): memory recurrence: gam = sigmoid(W2 relu(u + Gmem@mem) + b2),
          mem = gam1*mem + gam2*cHat
  ph4: output MLP on [h_l, h_a, h_v, mem]

All activations feature-major [features(partitions), batch(free)].
Weights are bf16 (stationary), activations fp32 moved as f32r.
"""
import numpy as np
import ml_dtypes

import concourse.bass as bass
import concourse.tile as tile
from concourse import bacc, mybir
from concourse.bass_utils import run_bass_kernel_spmd

F32 = mybir.dt.float32
F32R = mybir.dt.float32r
BF16 = mybir.dt.bfloat16
AF = mybir.ActivationFunctionType

# Problem constants (kernel.py must be self-contained)
T_FULL = 512
NBATCH = 256
NCORES = 8
B = NBATCH // NCORES          # 32 batch rows per core
D_L, D_A, D_V = 300, 74, 35
DIN = D_L + D_A + D_V         # 409
DH = 128
MEM = 256
CH = 8                        # timesteps per chunk
NB = CH * B                   # 256 free columns per chunk

# gate slot order: s = g'*3 + m, g' in (i, f, o, g_tanh); torch rows are (i, f, g, o)
TORCH_G = (0, 1, 3, 2)

# x feature chunks (K-dim pieces for ph0): (row0, rows, modality)
XCHUNKS = [(0, 128, 0), (128, 128, 0), (256, 44, 0), (300, 74, 1), (374, 35, 2)]
# modality -> list of x-chunk ids
M_KCS = {0: [0, 1, 2], 1: [3], 2: [4]}


def build_program(Tp=T_FULL):
    assert Tp % CH == 0
    NCH = Tp // CH
    nc = bacc.Bacc("TRN2", target_bir_lowering=False, debug=False)

    # ---------------- external inputs ----------------
    xT = nc.dram_tensor("xT", [DIN, Tp * B], F32, kind="ExternalInput")
    wih5 = nc.dram_tensor("wih5", [5 * 128, 1536], F32, kind="ExternalInput")
    whhT = nc.dram_tensor("whhT", [128, 1536], F32, kind="ExternalInput")
    bb12 = nc.dram_tensor("bb12", [128, 12], F32, kind="ExternalInput")
    ident = nc.dram_tensor("ident", [128, 128], F32, kind="ExternalInput")
    ones128 = nc.dram_tensor("ones128", [128, 1], F32, kind="ExternalInput")

    a1w1 = nc.dram_tensor("a1w1", [768, 256], F32, kind="ExternalInput")
    a1b1 = nc.dram_tensor("a1b1", [128, 2], F32, kind="ExternalInput")
    a1w2 = nc.dram_tensor("a1w2", [256, 768], F32, kind="ExternalInput")
    a1b2 = nc.dram_tensor("a1b2", [128, 6], F32, kind="ExternalInput")
    a2w1 = nc.dram_tensor("a2w1", [768, 256], F32, kind="ExternalInput")
    a2b1r = nc.dram_tensor("a2b1r", [1, 256], F32, kind="ExternalInput")
    a2w2 = nc.dram_tensor("a2w2", [256, 256], F32, kind="ExternalInput")
    a2b2c = nc.dram_tensor("a2b2c", [128, 2], F32, kind="ExternalInput")
    g1a = nc.dram_tensor("g1a", [768, 256], F32, kind="ExternalInput")
    g2a = nc.dram_tensor("g2a", [768, 256], F32, kind="ExternalInput")
    g1b1r = nc.dram_tensor("g1b1r", [1, 256], F32, kind="ExternalInput")
    g2b1r = nc.dram_tensor("g2b1r", [1, 256], F32, kind="ExternalInput")
    g1b = nc.dram_tensor("g1b", [256, 256], F32, kind="ExternalInput")
    g2b = nc.dram_tensor("g2b", [256, 256], F32, kind="ExternalInput")
    g1w2 = nc.dram_tensor("g1w2", [256, 256], F32, kind="ExternalInput")
    g2w2 = nc.dram_tensor("g2w2", [256, 256], F32, kind="ExternalInput")
    gb2r = nc.dram_tensor("gb2r", [1, 512], F32, kind="ExternalInput")
    ow1 = nc.dram_tensor("ow1", [640, 256], F32, kind="ExternalInput")
    ob1 = nc.dram_tensor("ob1", [128, 2], F32, kind="ExternalInput")
    ow2 = nc.dram_tensor("ow2", [256, 1], F32, kind="ExternalInput")
    ob2 = nc.dram_tensor("ob2", [1, 1], F32, kind="ExternalInput")

    out_d = nc.dram_tensor("out", [B, 1], F32, kind="ExternalOutput")

    import contextlib
    with tile.TileContext(nc) as tc:
        ctx = contextlib.ExitStack()
        with ctx:
            wpool = ctx.enter_context(tc.tile_pool(name="weights", bufs=1))
            xpool = ctx.enter_context(tc.tile_pool(name="xin", bufs=2))
            xwbpool = ctx.enter_context(tc.tile_pool(name="xwb", bufs=2))
            cpool = ctx.enter_context(tc.tile_pool(name="cring", bufs=4))
            upool = ctx.enter_context(tc.tile_pool(name="uring", bufs=3))
            chpool = ctx.enter_context(tc.tile_pool(name="chring", bufs=3))
            apool = ctx.enter_context(tc.tile_pool(name="attn", bufs=2))
            s1pool = ctx.enter_context(tc.tile_pool(name="sc1", bufs=2))
            s3pool = ctx.enter_context(tc.tile_pool(name="sc3", bufs=2))
            hpool = ctx.enter_context(tc.tile_pool(name="hstate", bufs=2))
            mpool = ctx.enter_context(tc.tile_pool(name="mstate", bufs=2))
            ppg = ctx.enter_context(tc.tile_pool(name="ppgates", bufs=2, space="PSUM"))
            pp3 = ctx.enter_context(tc.tile_pool(name="pp3", bufs=2, space="PSUM"))
            ppb = ctx.enter_context(tc.tile_pool(name="ppbig", bufs=3, space="PSUM"))
            pp0 = ctx.enter_context(tc.tile_pool(name="pp0", bufs=1, space="PSUM"))

            # ---- resident weights / constants ----
            wih5_t = wpool.tile([128, 5, 1536], F32R)
            nc.sync.dma_start(
                wih5_t[:], wih5.ap().rearrange("(kc p) c -> p kc c", p=128).bitcast(F32R))
            whhT_t = wpool.tile([128, 1536], F32R)
            nc.sync.dma_start(whhT_t[:], whhT.ap().bitcast(F32R))
            bb12_t = wpool.tile([128, 12], F32)
            nc.sync.dma_start(bb12_t[:], bb12.ap())
            id_t = wpool.tile([128, 128], F32R)
            nc.sync.dma_start(id_t[:], ident.ap().bitcast(F32R))
            ones128_t = wpool.tile([128, 1], F32R)
            nc.sync.dma_start(ones128_t[:], ones128.ap().bitcast(F32R))

            a1w1_t = wpool.tile([128, 6, 256], F32R)
            nc.sync.dma_start(a1w1_t[:], a1w1.ap().rearrange("(kc p) c -> p kc c", p=128).bitcast(F32R))
            a1b1_t = wpool.tile([128, 2], F32)
            nc.sync.dma_start(a1b1_t[:], a1b1.ap())
            a1w2_t = wpool.tile([128, 2, 768], F32R)
            nc.sync.dma_start(a1w2_t[:], a1w2.ap().rearrange("(kc p) c -> p kc c", p=128).bitcast(F32R))
            a1b2_t = wpool.tile([128, 6], F32)
            nc.sync.dma_start(a1b2_t[:], a1b2.ap())
            a2w1_t = wpool.tile([128, 6, 256], F32R)
            nc.sync.dma_start(a2w1_t[:], a2w1.ap().rearrange("(kc p) c -> p kc c", p=128).bitcast(F32R))
            a2b1r_t = wpool.tile([1, 256], F32R)
            nc.sync.dma_start(a2b1r_t[:], a2b1r.ap().bitcast(F32R))
            a2w2_t = wpool.tile([128, 2, 256], F32R)
            nc.sync.dma_start(a2w2_t[:], a2w2.ap().rearrange("(kc p) c -> p kc c", p=128).bitcast(F32R))
            a2b2c_t = wpool.tile([128, 2], F32)
            nc.sync.dma_start(a2b2c_t[:], a2b2c.ap())
            g1a_t = wpool.tile([128, 6, 256], F32R)
            nc.sync.dma_start(g1a_t[:], g1a.ap().rearrange("(kc p) c -> p kc c", p=128).bitcast(F32R))
            g2a_t = wpool.tile([128, 6, 256], F32R)
            nc.sync.dma_start(g2a_t[:], g2a.ap().rearrange("(kc p) c -> p kc c", p=128).bitcast(F32R))
            g1b1r_t = wpool.tile([1, 256], F32R)
            nc.sync.dma_start(g1b1r_t[:], g1b1r.ap().bitcast(F32R))
            g2b1r_t = wpool.tile([1, 256], F32R)
            nc.sync.dma_start(g2b1r_t[:], g2b1r.ap().bitcast(F32R))
            g1b_t = wpool.tile([128, 2, 256], F32R)
            nc.sync.dma_start(g1b_t[:], g1b.ap().rearrange("(kc p) c -> p kc c", p=128).bitcast(F32R))
            g2b_t = wpool.tile([128, 2, 256], F32R)
            nc.sync.dma_start(g2b_t[:], g2b.ap().rearrange("(kc p) c -> p kc c", p=128).bitcast(F32R))
            g1w2_t = wpool.tile([128, 2, 256], F32R)
            nc.sync.dma_start(g1w2_t[:], g1w2.ap().rearrange("(kc p) c -> p kc c", p=128).bitcast(F32R))
            g2w2_t = wpool.tile([128, 2, 256], F32R)
            nc.sync.dma_start(g2w2_t[:], g2w2.ap().rearrange("(kc p) c -> p kc c", p=128).bitcast(F32R))
            gb2r_t = wpool.tile([1, 512], F32R)
            nc.sync.dma_start(gb2r_t[:], gb2r.ap().bitcast(F32R))
            ow1_t = wpool.tile([128, 5, 256], F32R)
            nc.sync.dma_start(ow1_t[:], ow1.ap().rearrange("(kc p) c -> p kc c", p=128).bitcast(F32R))
            ob1_t = wpool.tile([128, 2], F32)
            nc.sync.dma_start(ob1_t[:], ob1.ap())
            ow2_t = wpool.tile([128, 2, 1], F32R)
            nc.sync.dma_start(ow2_t[:], ow2.ap().rearrange("(kc p) c -> p kc c", p=128).bitcast(F32R))
            ob2_t = wpool.tile([1, 1], F32)
            nc.sync.dma_start(ob2_t[:], ob2.ap())

            onesb_t = wpool.tile([1, 32], F32R)
            nc.vector.memset(onesb_t[:].bitcast(F32), 1.0)
            onesr_t = wpool.tile([1, 128], F32R)
            nc.vector.memset(onesr_t[:].bitcast(F32), 1.0)

            h_cur = hpool.tile([128, 3, 32], F32R, tag="h")
            nc.vector.memset(h_cur[:].bitcast(F32), 0.0)
            mem_cur = mpool.tile([128, 2, 32], F32R, tag="mem")
            nc.vector.memset(mem_cur[:].bitcast(F32), 0.0)

            cring_prev = None

            for w in range(NCH):
                t0 = w * CH
                c0 = t0 * B

                # =========== ph0(w): x load + xwb matmuls ===========
                xa = xpool.tile([128, 5, NB], F32R, tag="x", name="xa")
                for kc, (r0, rows, m) in enumerate(XCHUNKS):
                    nc.sync.dma_start(
                        xa[0:rows, kc, :], xT.ap()[r0:r0 + rows, c0:c0 + NB].bitcast(F32R))

                xwb = xwbpool.tile([128, 12, CH, 32], F32R, tag="xwb", name="xwb")
                for sp in range(6):  # slot pairs
                    pt = pp0.tile([128, 2, NB], F32, tag="pt", name="pt")
                    for i in range(2):
                        s = sp * 2 + i
                        m = s % 3
                        kcs = M_KCS[m]
                        for ki, kc in enumerate(kcs):
                            rows = XCHUNKS[kc][1]
                            nc.tensor.matmul(
                                pt[:, i, :],
                                wih5_t[0:rows, kc, s * 128:(s + 1) * 128],
                                xa[0:rows, kc, :],
                                start=(ki == 0), stop=(ki == len(kcs) - 1))
                        nc.scalar.activation(
                            xwb[:, s, :, :].rearrange("p t b -> p (t b)"),
                            pt[:, i, :], AF.Identity, bias=bb12_t[:, s:s + 1])

                # =========== ph1(w): LSTM recurrence ===========
                cring = cpool.tile([128, 3, CH + 1, 32], F32R, tag="c", name="cring")
                if w == 0:
                    nc.vector.memset(cring[:, :, 0, :].bitcast(F32), 0.0)
                else:
                    nc.vector.tensor_copy(cring[:, :, 0, :], cring_prev[:, :, CH, :])

                for j in range(CH):
                    gates = ppg.tile([128, 12, 32], F32, tag="gates", name="gates")
                    nc.tensor.matmul(gates[:], id_t[:], xwb[:, :, j, :],
                                     start=True, stop=False)
                    for s in range(12):
                        m = s % 3
                        nc.tensor.matmul(
                            gates[:, s, :],
                            whhT_t[:, s * 128:(s + 1) * 128],
                            h_cur[:, m, :],
                            start=False, stop=(s == 11))
                    sg = s1pool.tile([128, 9, 32], F32, tag="sg")
                    nc.scalar.activation(sg[:], gates[:, 0:9, :], AF.Sigmoid)
                    tg = s1pool.tile([128, 3, 32], F32, tag="tg")
                    nc.scalar.activation(tg[:], gates[:, 9:12, :], AF.Tanh)
                    t1 = s1pool.tile([128, 3, 32], F32, tag="t1")
                    nc.vector.tensor_mul(t1[:], sg[:, 0:3, :], tg[:])
                    t2 = s1pool.tile([128, 3, 32], F32, tag="t2")
                    nc.vector.tensor_mul(t2[:], sg[:, 3:6, :], cring[:, :, j, :].bitcast(F32))
                    nc.vector.tensor_add(cring[:, :, j + 1, :], t1[:], t2[:])
                    tc_t = s1pool.tile([128, 3, 32], F32, tag="tc")
                    nc.scalar.activation(tc_t[:], cring[:, :, j + 1, :].bitcast(F32), AF.Tanh)
                    h_new = hpool.tile([128, 3, 32], F32R, tag="h", name="h_new")
                    nc.vector.tensor_mul(h_new[:], sg[:, 6:9, :], tc_t[:])
                    h_cur = h_new
                cring_prev = cring

                # =========== ph2a(w): time-parallel attention ===========
                def cstar(q):
                    # q 0..2: prev c (t0-1..t0+6); q 3..5: new c (t0..t0+7)
                    if q < 3:
                        return cring[:, q, 0:CH, :]
                    return cring[:, q - 3, 1:CH + 1, :]

                # att1 layer 1 + relu
                y1p = ppb.tile([128, 2, NB], F32, tag="big", name="y1p")
                for mc in range(2):
                    for kc in range(6):
                        nc.tensor.matmul(
                            y1p[:, mc, :], a1w1_t[:, kc, mc * 128:(mc + 1) * 128],
                            cstar(kc), start=(kc == 0), stop=(kc == 5))
                y1 = apool.tile([128, 2, NB], F32R, tag="y1")
                for mc in range(2):
                    nc.scalar.activation(y1[:, mc, :], y1p[:, mc, :], AF.Relu,
                                         bias=a1b1_t[:, mc:mc + 1])
                # att1 layer 2 + exp
                et = apool.tile([128, 6, CH, 32], F32R, tag="et")
                for qp in range(3):
                    ep = ppb.tile([128, 2, NB], F32, tag="big", name="ep")
                    for i in range(2):
                        q = qp * 2 + i
                        for kc in range(2):
                            nc.tensor.matmul(
                                ep[:, i, :], a1w2_t[:, kc, q * 128:(q + 1) * 128],
                                y1[:, kc, :],
                                start=(kc == 0), stop=(kc == 1))
                        nc.scalar.activation(
                            et[:, q, :, :].rearrange("p t b -> p (t b)"),
                            ep[:, i, :], AF.Exp, bias=a1b2_t[:, q:q + 1])
                # S = colsum(E), srow/sinv, broadcast
                ssb = ppb.tile([128, 2, NB], F32, tag="big", name="ssb")
                for q in range(6):
                    nc.tensor.matmul(
                        ssb[0:1, 1, :], ones128_t[:],
                        et[:, q, :, :].rearrange("p t b -> p (t b)"),
                        start=(q == 0), stop=(q == 5))
                srow = apool.tile([1, NB], F32R, tag="srow")
                sinv = apool.tile([1, NB], F32R, tag="sinv")
                with nc.allow_low_precision(reason="f32r rounding of softmax sum is benign"):
                    nc.vector.tensor_copy(srow[:], ssb[0:1, 1, :])
                    nc.vector.reciprocal(sinv[:], ssb[0:1, 1, :])
                nc.tensor.matmul(ssb[:, 0, :], onesr_t[:], sinv[:],
                                 start=True, stop=True)
                sb = apool.tile([128, NB], F32, tag="sb")
                nc.scalar.activation(sb[:], ssb[:, 0, :], AF.Identity)

                # U = E * cStar (unnormalized)
                ut = apool.tile([128, 6, CH, 32], F32R, tag="ut")
                nc.vector.tensor_mul(ut[:, 0:3, :, :], et[:, 0:3, :, :].bitcast(F32),
                                     cring[:, :, 0:CH, :].bitcast(F32))
                nc.vector.tensor_mul(ut[:, 3:6, :, :], et[:, 3:6, :, :].bitcast(F32),
                                     cring[:, :, 1:CH + 1, :].bitcast(F32))

                sb3 = sb[:].unsqueeze(1).broadcast_to([128, 2, NB])

                # att2 layer 1: z = relu((a2w1@U + b1*S) * sinv)
                zp = ppb.tile([128, 2, NB], F32, tag="big", name="zp")
                for mc in range(2):
                    nc.tensor.matmul(zp[:, mc, :], a2b1r_t[:, mc * 128:(mc + 1) * 128],
                                     srow[:], start=True, stop=False)
                    for kc in range(6):
                        nc.tensor.matmul(
                            zp[:, mc, :], a2w1_t[:, kc, mc * 128:(mc + 1) * 128],
                            ut[:, kc, :, :].rearrange("p t b -> p (t b)"),
                            start=False, stop=(kc == 5))
                zn = apool.tile([128, 2, NB], F32, tag="zn")
                nc.vector.tensor_mul(zn[:], zp[:], sb3)
                z = apool.tile([128, 2, NB], F32R, tag="z")
                nc.scalar.activation(z[:], zn[:], AF.Relu)

                # att2 layer 2 + tanh -> cHat ring
                chring = chpool.tile([128, 2, CH, 32], F32, tag="ch", name="chring")
                ap2 = ppb.tile([128, 2, NB], F32, tag="big", name="ap2")
                for mc in range(2):
                    for kc in range(2):
                        nc.tensor.matmul(
                            ap2[:, mc, :], a2w2_t[:, kc, mc * 128:(mc + 1) * 128],
                            z[:, kc, :], start=(kc == 0), stop=(kc == 1))
                    nc.scalar.activation(
                        chring[:, mc, :, :].rearrange("p t b -> p (t b)"),
                        ap2[:, mc, :], AF.Tanh, bias=a2b2c_t[:, mc:mc + 1])

                # g1/g2 attended part, normalized: u = (ga@U + b1*S) * sinv
                uring = upool.tile([128, 4, CH, 32], F32, tag="u", name="uring")
                for gi, (gw, gbr) in enumerate(((g1a_t, g1b1r_t), (g2a_t, g2b1r_t))):
                    gp = ppb.tile([128, 2, NB], F32, tag="big", name="gp")
                    for mc in range(2):
                        nc.tensor.matmul(gp[:, mc, :], gbr[:, mc * 128:(mc + 1) * 128],
                                         srow[:], start=True, stop=False)
                        for kc in range(6):
                            nc.tensor.matmul(
                                gp[:, mc, :], gw[:, kc, mc * 128:(mc + 1) * 128],
                                ut[:, kc, :, :].rearrange("p t b -> p (t b)"),
                                start=False, stop=(kc == 5))
                    nc.vector.tensor_mul(
                        uring[:, 2 * gi:2 * gi + 2, :, :].rearrange("p a t b -> p a (t b)"),
                        gp[:], sb3)

                # =========== ph3(w): memory recurrence ===========
                for j in range(CH):
                    p3 = pp3.tile([128, 8, 32], F32, tag="p3", name="p3")
                    # qg bias preload (independent of hh)
                    for r in range(4):
                        nc.tensor.matmul(p3[:, 4 + r, :],
                                         gb2r_t[:, r * 128:(r + 1) * 128],
                                         onesb_t[:], start=True, stop=False)
                    # pg = Gmem @ mem
                    for r, gwt in enumerate((g1b_t, g1b_t, g2b_t, g2b_t)):
                        mc = r % 2
                        for kc in range(2):
                            nc.tensor.matmul(
                                p3[:, r, :], gwt[:, kc, mc * 128:(mc + 1) * 128],
                                mem_cur[:, kc, :],
                                start=(kc == 0), stop=(kc == 1))
                    wv = s3pool.tile([128, 4, 32], F32, tag="wv")
                    nc.vector.tensor_add(wv[:], p3[:, 0:4, :], uring[:, :, j, :])
                    hh = s3pool.tile([128, 4, 32], F32R, tag="hh")
                    nc.vector.tensor_scalar_max(hh[:], wv[:], 0.0)
                    # qg += W2 @ hh
                    for r, gwt in enumerate((g1w2_t, g1w2_t, g2w2_t, g2w2_t)):
                        mc = r % 2
                        goff = 0 if r < 2 else 2
                        for kc in range(2):
                            nc.tensor.matmul(
                                p3[:, 4 + r, :], gwt[:, kc, mc * 128:(mc + 1) * 128],
                                hh[:, goff + kc, :],
                                start=False, stop=(kc == 1))
                    gam = s3pool.tile([128, 4, 32], F32, tag="gam")
                    nc.scalar.activation(gam[:], p3[:, 4:8, :], AF.Sigmoid)
                    m1 = s3pool.tile([128, 2, 32], F32, tag="m1")
                    nc.vector.tensor_mul(m1[:], gam[:, 0:2, :], mem_cur[:].bitcast(F32))
                    m2 = s3pool.tile([128, 2, 32], F32, tag="m2")
                    nc.vector.tensor_mul(m2[:], gam[:, 2:4, :], chring[:, :, j, :])
                    mem_new = mpool.tile([128, 2, 32], F32R, tag="mem", name="mem_new")
                    nc.vector.tensor_add(mem_new[:], m1[:], m2[:])
                    mem_cur = mem_new

            # =========== ph4: output MLP ===========
            o1p = ppb.tile([128, 2, 32], F32, tag="big", name="o1p")
            rhs5 = [h_cur[:, 0, :], h_cur[:, 1, :], h_cur[:, 2, :],
                    mem_cur[:, 0, :], mem_cur[:, 1, :]]
            for mc in range(2):
                for kc in range(5):
                    nc.tensor.matmul(
                        o1p[:, mc, :], ow1_t[:, kc, mc * 128:(mc + 1) * 128],
                        rhs5[kc], start=(kc == 0), stop=(kc == 4))
            o1s = s3pool.tile([128, 2, 32], F32R, tag="o1s")
            for mc in range(2):
                nc.scalar.activation(o1s[:, mc, :], o1p[:, mc, :], AF.Relu,
                                     bias=ob1_t[:, mc:mc + 1])
            o2p = ppb.tile([1, 32], F32, tag="big", name="o2p")
            for kc in range(2):
                nc.tensor.matmul(o2p[:], ow2_t[:, kc, :], o1s[:, kc, :],
                                 start=(kc == 0), stop=(kc == 1))
            o2s = s3pool.tile([1, 32], F32, tag="o2s")
            nc.scalar.activation(o2s[:], o2p[:], AF.Identity, bias=ob2_t[:])
            nc.sync.dma_start(out_d.ap().rearrange("b one -> (one) (b)"), o2s[:])

    nc.compile()
    return nc


# ---------------------------------------------------------------------------
# host-side packing
# ---------------------------------------------------------------------------

def _bf(x):
    return np.ascontiguousarray(np.asarray(x, np.float32))


def pack_shared(inp):
    """Pack weight tensors (identical across cores)."""
    f = np.float32
    d = {}
    wih = {0: inp["Wih_l"], 1: inp["Wih_a"], 2: inp["Wih_v"]}
    whh = {0: inp["Whh_l"], 1: inp["Whh_a"], 2: inp["Whh_v"]}
    bb = {m: (np.asarray(inp[f"bih_{k}"], f) + np.asarray(inp[f"bhh_{k}"], f))
          for m, k in ((0, "l"), (1, "a"), (2, "v"))}
    foff = {0: 0, 1: D_L, 2: D_L + D_A}

    wih5 = np.zeros((5 * 128, 1536), f)
    whhT = np.zeros((128, 1536), f)
    bb12 = np.zeros((128, 12), f)
    for gq in range(4):
        tg = TORCH_G[gq]
        for m in range(3):
            s = gq * 3 + m
            wblk = np.asarray(wih[m], f)[tg * 128:(tg + 1) * 128, :]  # [128, din]
            whhT[:, s * 128:(s + 1) * 128] = np.asarray(whh[m], f)[tg * 128:(tg + 1) * 128, :].T
            bb12[:, s] = bb[m][tg * 128:(tg + 1) * 128]
            # scatter wblk.T rows (x-features of modality m) into the 5-chunk layout
            for kc, (r0, rows, km) in enumerate(XCHUNKS):
                if km != m:
                    continue
                loc0 = r0 - foff[m]  # feature offset within modality
                wih5[kc * 128:kc * 128 + rows, s * 128:(s + 1) * 128] = \
                    wblk[:, loc0:loc0 + rows].T
    d["wih5"] = _bf(wih5)
    d["whhT"] = _bf(whhT)
    d["bb12"] = bb12
    d["ident"] = _bf(np.eye(128, dtype=f))
    d["ones128"] = _bf(np.ones((128, 1), f))

    d["a1w1"] = _bf(np.asarray(inp["att1_W1"], f).T)              # [768, 256]
    d["a1b1"] = np.asarray(inp["att1_b1"], f).reshape(2, 128).T.copy()
    d["a1w2"] = _bf(np.asarray(inp["att1_W2"], f).T)              # [256, 768]
    d["a1b2"] = np.asarray(inp["att1_b2"], f).reshape(6, 128).T.copy()
    d["a2w1"] = _bf(np.asarray(inp["att2_W1"], f).T)
    d["a2b1r"] = _bf(np.asarray(inp["att2_b1"], f).reshape(1, 256))
    d["a2w2"] = _bf(np.asarray(inp["att2_W2"], f).T)              # [256, 256]
    d["a2b2c"] = np.asarray(inp["att2_b2"], f).reshape(2, 128).T.copy()
    d["g1a"] = _bf(np.asarray(inp["g1_W1"], f)[:, :768].T)
    d["g2a"] = _bf(np.asarray(inp["g2_W1"], f)[:, :768].T)
    d["g1b"] = _bf(np.asarray(inp["g1_W1"], f)[:, 768:].T)
    d["g2b"] = _bf(np.asarray(inp["g2_W1"], f)[:, 768:].T)
    d["g1b1r"] = _bf(np.asarray(inp["g1_b1"], f).reshape(1, 256))
    d["g2b1r"] = _bf(np.asarray(inp["g2_b1"], f).reshape(1, 256))
    d["g1w2"] = _bf(np.asarray(inp["g1_W2"], f).T)
    d["g2w2"] = _bf(np.asarray(inp["g2_W2"], f).T)
    d["gb2r"] = _bf(np.concatenate([np.asarray(inp["g1_b2"], f),
                                    np.asarray(inp["g2_b2"], f)]).reshape(1, 512))
    d["ow1"] = _bf(np.asarray(inp["out_W1"], f).T)                # [640, 256]
    d["ob1"] = np.asarray(inp["out_b1"], f).reshape(2, 128).T.copy()
    d["ow2"] = _bf(np.asarray(inp["out_W2"], f).T)                # [256, 1]
    d["ob2"] = np.asarray(inp["out_b2"], f).reshape(1, 1).copy()
    return d


def pack_x(x, core, Tp):
    """x: [Tp, 256, 409] -> xT [409, Tp*B] for one core."""
    xc = np.asarray(x[:, core * B:(core + 1) * B, :], np.float32)   # [Tp, B, 409]
    return np.ascontiguousarray(xc.transpose(2, 0, 1).reshape(DIN, Tp * B))


_CACHE = {}


def _get_program(Tp):
    if Tp not in _CACHE:
        _CACHE[Tp] = build_program(Tp)
    return _CACHE[Tp]


def kernel(**inputs):
    x = np.asarray(inputs["x"])
    Tp = x.shape[0]
    nc = _get_program(Tp)
    shared = pack_shared(inputs)
    in_maps = []
    for c in range(NCORES):
        m = dict(shared)
        m["xT"] = pack_x(x, c, Tp)
        in_maps.append(m)
    res = run_bass_kernel_spmd(nc, in_maps, list(range(NCORES))).results
    out = np.concatenate([r["out"] for r in res], axis=0)
    return out.astype(np.float32)


if __name__ == "__main__":
    import time
    t0 = time.time()
    nc = build_program(32)
    print("built in", time.time() - t0, "s")


# revision 24
# speedup vs baseline: 3.5125x; 1.7505x over previous
"""Trainium2 Bass kernel for nn_Contextual_MFN (Memory Fusion Network).

Fully-fused single-TileContext design (v2). Batch data-parallel over 8
cores (32 rows/core). Per chunk of CH=8 timesteps, four pipeline stages
flow through SBUF rings with no DRAM round-trips; the tile scheduler
overlaps them across chunks:

  ph0(w): xwb = Wih @ x chunk (time-parallel matmuls, bias via ACT copy)
  ph1(w): 3xLSTM recurrence; gates = inject(xwb) + Whh @ h per step
  ph2a(w): time-parallel attention on the c-sequence; produces
           normalized gate pre-activations u (attended part + b1),
           cHat = tanh(att2 MLP) -- everything not mem-dependent
  ph3(w): memory recurrence: gam = sigmoid(W2 relu(u + Gmem@mem) + b2),
          mem = gam1*mem + gam2*cHat
  ph4: output MLP on [h_l, h_a, h_v, mem]

All activations feature-major [features(partitions), batch(free)].
Weights are bf16 (stationary), activations fp32 moved as f32r.
"""
import numpy as np
import ml_dtypes

import concourse.bass as bass
import concourse.tile as tile
from concourse import bacc, mybir
from concourse.bass_utils import run_bass_kernel_spmd

F32 = mybir.dt.float32
F32R = mybir.dt.float32r
BF16 = mybir.dt.bfloat16
AF = mybir.ActivationFunctionType

# Problem constants (kernel.py must be self-contained)
T_FULL = 512
NBATCH = 256
NCORES = 8
B = NBATCH // NCORES          # 32 batch rows per core
D_L, D_A, D_V = 300, 74, 35
DIN = D_L + D_A + D_V         # 409
DH = 128
MEM = 256
CH = 8                        # timesteps per chunk
NB = CH * B                   # 256 free columns per chunk

# gate slot order: s = g'*3 + m, g' in (i, f, o, g_tanh); torch rows are (i, f, g, o)
TORCH_G = (0, 1, 3, 2)

# x feature chunks (K-dim pieces for ph0): (row0, rows, modality)
XCHUNKS = [(0, 128, 0), (128, 128, 0), (256, 44, 0), (300, 74, 1), (374, 35, 2)]
# modality -> list of x-chunk ids
M_KCS = {0: [0, 1, 2], 1: [3], 2: [4]}


DBG = False
PHASES = 4  # debug: 0=ph0 only, 1=+ph1, 2=+2a, 3=+ph3


def build_program(Tp=T_FULL):
    assert Tp % CH == 0
    NCH = Tp // CH
    nc = bacc.Bacc("TRN2", target_bir_lowering=False, debug=False)

    # ---------------- external inputs ----------------
    xT = nc.dram_tensor("xT", [DIN, Tp * B], BF16, kind="ExternalInput")
    wih5 = nc.dram_tensor("wih5", [5 * 128, 1536], BF16, kind="ExternalInput")
    whhT = nc.dram_tensor("whhT", [128, 1536], BF16, kind="ExternalInput")
    bb12 = nc.dram_tensor("bb12", [128, 12], F32, kind="ExternalInput")
    ident = nc.dram_tensor("ident", [128, 128], F32, kind="ExternalInput")
    ones128 = nc.dram_tensor("ones128", [128, 1], BF16, kind="ExternalInput")

    a1w1 = nc.dram_tensor("a1w1", [768, 256], F32, kind="ExternalInput")
    a1b1 = nc.dram_tensor("a1b1", [128, 2], F32, kind="ExternalInput")
    a1w2 = nc.dram_tensor("a1w2", [256, 768], BF16, kind="ExternalInput")
    a1b2 = nc.dram_tensor("a1b2", [128, 6], F32, kind="ExternalInput")
    a2w1 = nc.dram_tensor("a2w1", [768, 256], BF16, kind="ExternalInput")
    a2b1r = nc.dram_tensor("a2b1r", [1, 256], BF16, kind="ExternalInput")
    a2w2 = nc.dram_tensor("a2w2", [256, 256], BF16, kind="ExternalInput")
    a2b2c = nc.dram_tensor("a2b2c", [128, 2], F32, kind="ExternalInput")
    g1a = nc.dram_tensor("g1a", [768, 256], BF16, kind="ExternalInput")
    g2a = nc.dram_tensor("g2a", [768, 256], BF16, kind="ExternalInput")
    g1b1r = nc.dram_tensor("g1b1r", [1, 256], BF16, kind="ExternalInput")
    g2b1r = nc.dram_tensor("g2b1r", [1, 256], BF16, kind="ExternalInput")
    g1b = nc.dram_tensor("g1b", [256, 256], F32, kind="ExternalInput")
    g2b = nc.dram_tensor("g2b", [256, 256], F32, kind="ExternalInput")
    g1w2 = nc.dram_tensor("g1w2", [256, 256], BF16, kind="ExternalInput")
    g2w2 = nc.dram_tensor("g2w2", [256, 256], BF16, kind="ExternalInput")
    gb2r = nc.dram_tensor("gb2r", [1, 512], BF16, kind="ExternalInput")
    ow1h = nc.dram_tensor("ow1h", [384, 256], BF16, kind="ExternalInput")
    ow1m = nc.dram_tensor("ow1m", [256, 256], F32, kind="ExternalInput")
    ob1 = nc.dram_tensor("ob1", [128, 2], F32, kind="ExternalInput")
    ow2 = nc.dram_tensor("ow2", [128, 2], F32, kind="ExternalInput")

    out_d = nc.dram_tensor("out", [B, 1], F32, kind="ExternalOutput")
    if DBG:
        NCHd = Tp // CH
        xwb_dbg = nc.dram_tensor("xwb_dbg", [NCHd, 128, 12, CH, 32], F32, kind="ExternalOutput")
        c_dbg = nc.dram_tensor("c_dbg", [NCHd, 128, 3, CH + 1, 32], F32, kind="ExternalOutput")
        h_dbg = nc.dram_tensor("h_dbg", [NCHd, 128, 3, 32], BF16, kind="ExternalOutput")
        u_dbg = nc.dram_tensor("u_dbg", [NCHd, 128, 4, CH, 32], F32, kind="ExternalOutput")
        ch_dbg = nc.dram_tensor("ch_dbg", [NCHd, 128, 2, CH, 32], F32, kind="ExternalOutput")
        et_dbg = nc.dram_tensor("et_dbg", [NCHd, 128, 6, CH, 32], BF16, kind="ExternalOutput")
        mem_dbg = nc.dram_tensor("mem_dbg", [NCHd, 128, 2, 32], F32, kind="ExternalOutput")
        xa_dbg = nc.dram_tensor("xa_dbg", [NCHd, 128, 5, 256], BF16, kind="ExternalOutput")

    import contextlib
    with tile.TileContext(nc) as tc:
        ctx = contextlib.ExitStack()
        with ctx:
            wpool = ctx.enter_context(tc.tile_pool(name="weights", bufs=1))
            xpool = ctx.enter_context(tc.tile_pool(name="xin", bufs=2))
            xwbpool = ctx.enter_context(tc.tile_pool(name="xwb", bufs=2))
            cpool = ctx.enter_context(tc.tile_pool(name="cring", bufs=4))
            upool = ctx.enter_context(tc.tile_pool(name="uring", bufs=3))
            chpool = ctx.enter_context(tc.tile_pool(name="chring", bufs=3))
            apool = ctx.enter_context(tc.tile_pool(name="attn", bufs=2))
            s1pool = ctx.enter_context(tc.tile_pool(name="sc1", bufs=2))
            s3pool = ctx.enter_context(tc.tile_pool(name="sc3", bufs=2))
            hpool = ctx.enter_context(tc.tile_pool(name="hstate", bufs=2))
            mpool = ctx.enter_context(tc.tile_pool(name="mstate", bufs=2))
            ppg = ctx.enter_context(tc.tile_pool(name="ppgates", bufs=2, space="PSUM"))
            pp3 = ctx.enter_context(tc.tile_pool(name="pp3", bufs=2, space="PSUM"))
            ppb = ctx.enter_context(tc.tile_pool(name="ppbig", bufs=3, space="PSUM"))
            pp0 = ctx.enter_context(tc.tile_pool(name="pp0", bufs=1, space="PSUM"))

            # ---- resident weights / constants ----
            wih5_t = wpool.tile([128, 5, 1536], BF16)
            nc.sync.dma_start(
                wih5_t[:], wih5.ap().rearrange("(kc p) c -> p kc c", p=128))
            whhT_t = wpool.tile([128, 1536], BF16)
            nc.sync.dma_start(whhT_t[:], whhT.ap())
            bb12_t = wpool.tile([128, 12], F32)
            nc.sync.dma_start(bb12_t[:], bb12.ap())
            id_t = wpool.tile([128, 128], F32R)
            nc.sync.dma_start(id_t[:], ident.ap().bitcast(F32R))
            ones128_t = wpool.tile([128, 1], BF16)
            nc.sync.dma_start(ones128_t[:], ones128.ap())

            a1w1_t = wpool.tile([128, 6, 256], F32R)
            nc.sync.dma_start(a1w1_t[:], a1w1.ap().rearrange("(kc p) c -> p kc c", p=128).bitcast(F32R))
            a1b1_t = wpool.tile([128, 2], F32)
            nc.sync.dma_start(a1b1_t[:], a1b1.ap())
            a1w2_t = wpool.tile([128, 2, 768], BF16)
            nc.sync.dma_start(a1w2_t[:], a1w2.ap().rearrange("(kc p) c -> p kc c", p=128))
            a1b2_t = wpool.tile([128, 6], F32)
            nc.sync.dma_start(a1b2_t[:], a1b2.ap())
            a2w1_t = wpool.tile([128, 6, 256], BF16)
            nc.sync.dma_start(a2w1_t[:], a2w1.ap().rearrange("(kc p) c -> p kc c", p=128))
            a2b1r_t = wpool.tile([1, 256], BF16)
            nc.sync.dma_start(a2b1r_t[:], a2b1r.ap())
            a2w2_t = wpool.tile([128, 2, 256], BF16)
            nc.sync.dma_start(a2w2_t[:], a2w2.ap().rearrange("(kc p) c -> p kc c", p=128))
            a2b2c_t = wpool.tile([128, 2], F32)
            nc.sync.dma_start(a2b2c_t[:], a2b2c.ap())
            g1a_t = wpool.tile([128, 6, 256], BF16)
            nc.sync.dma_start(g1a_t[:], g1a.ap().rearrange("(kc p) c -> p kc c", p=128))
            g2a_t = wpool.tile([128, 6, 256], BF16)
            nc.sync.dma_start(g2a_t[:], g2a.ap().rearrange("(kc p) c -> p kc c", p=128))
            g1b1r_t = wpool.tile([1, 256], BF16)
            nc.sync.dma_start(g1b1r_t[:], g1b1r.ap())
            g2b1r_t = wpool.tile([1, 256], BF16)
            nc.sync.dma_start(g2b1r_t[:], g2b1r.ap())
            g1b_t = wpool.tile([128, 2, 256], F32R)
            nc.sync.dma_start(g1b_t[:], g1b.ap().rearrange("(kc p) c -> p kc c", p=128).bitcast(F32R))
            g2b_t = wpool.tile([128, 2, 256], F32R)
            nc.sync.dma_start(g2b_t[:], g2b.ap().rearrange("(kc p) c -> p kc c", p=128).bitcast(F32R))
            g1w2_t = wpool.tile([128, 2, 256], BF16)
            nc.sync.dma_start(g1w2_t[:], g1w2.ap().rearrange("(kc p) c -> p kc c", p=128))
            g2w2_t = wpool.tile([128, 2, 256], BF16)
            nc.sync.dma_start(g2w2_t[:], g2w2.ap().rearrange("(kc p) c -> p kc c", p=128))
            gb2r_t = wpool.tile([1, 512], BF16)
            nc.sync.dma_start(gb2r_t[:], gb2r.ap())
            ow1h_t = wpool.tile([128, 3, 256], BF16)
            nc.sync.dma_start(ow1h_t[:], ow1h.ap().rearrange("(kc p) c -> p kc c", p=128))
            ow1m_t = wpool.tile([128, 2, 256], F32R)
            nc.sync.dma_start(ow1m_t[:], ow1m.ap().rearrange("(kc p) c -> p kc c", p=128).bitcast(F32R))
            ob1_t = wpool.tile([128, 2], F32)
            nc.sync.dma_start(ob1_t[:], ob1.ap())
            ow2_t = wpool.tile([128, 2], F32R)
            nc.sync.dma_start(ow2_t[:], ow2.ap().bitcast(F32R))

            onesb_t = wpool.tile([1, 32], BF16)
            nc.vector.memset(onesb_t[:], 1.0)
            onesr_t = wpool.tile([1, 128], BF16)
            nc.vector.memset(onesr_t[:], 1.0)

            h_cur = hpool.tile([128, 3, 32], BF16, tag="h")
            nc.vector.memset(h_cur[:], 0.0)
            mem_cur = mpool.tile([128, 2, 32], F32R, tag="mem")
            nc.vector.memset(mem_cur[:].bitcast(F32), 0.0)

            cring_prev = None

            for w in range(NCH):
                t0 = w * CH
                c0 = t0 * B

                # =========== ph0(w): x load + xwb matmuls ===========
                xa = xpool.tile([128, 5, NB], BF16, tag="x", name="xa")
                for kc, (r0, rows, m) in enumerate(XCHUNKS):
                    nc.sync.dma_start(
                        xa[0:rows, kc, :], xT.ap()[r0:r0 + rows, c0:c0 + NB])

                xwb = xwbpool.tile([128, 12, CH, 32], F32R, tag="xwb", name="xwb")
                if PHASES < 0:
                    nc.vector.memset(xwb[:].bitcast(F32), 0.0)
                for sp in range(PHASES >= 0 and 6 or 0):  # slot pairs
                    pt = pp0.tile([128, 2, NB], F32, tag="pt", name="pt")
                    for i in range(2):
                        s = sp * 2 + i
                        m = s % 3
                        kcs = M_KCS[m]
                        for ki, kc in enumerate(kcs):
                            rows = XCHUNKS[kc][1]
                            nc.tensor.matmul(
                                pt[:, i, :],
                                wih5_t[0:rows, kc, s * 128:(s + 1) * 128],
                                xa[0:rows, kc, :],
                                start=(ki == 0), stop=(ki == len(kcs) - 1))
                        nc.scalar.activation(
                            xwb[:, s, :, :].rearrange("p t b -> p (t b)"),
                            pt[:, i, :], AF.Identity, bias=bb12_t[:, s:s + 1])

                if DBG:
                    nc.sync.dma_start(xwb_dbg.ap()[w], xwb[:].bitcast(F32))
                    nc.sync.dma_start(xa_dbg.ap()[w], xa[:])
                # =========== ph1(w): LSTM recurrence ===========
                if PHASES < 1:
                    continue
                cring = cpool.tile([128, 3, CH + 1, 32], F32R, tag="c", name="cring")
                if w == 0:
                    nc.vector.memset(cring[:, :, 0, :].bitcast(F32), 0.0)
                else:
                    nc.vector.tensor_copy(cring[:, :, 0, :], cring_prev[:, :, CH, :])

                for j in range(CH):
                    gates = ppg.tile([128, 12, 32], F32, tag="gates", name="gates")
                    nc.tensor.matmul(gates[:], id_t[:], xwb[:, :, j, :],
                                     start=True, stop=False)
                    for s in range(12):
                        m = s % 3
                        nc.tensor.matmul(
                            gates[:, s, :],
                            whhT_t[:, s * 128:(s + 1) * 128],
                            h_cur[:, m, :],
                            start=False, stop=(s == 11))
                    th = s1pool.tile([128, 12, 32], F32, tag="th")
                    nc.scalar.activation(th[:], gates[:], AF.Tanh)
                    t1 = s1pool.tile([128, 3, 32], F32, tag="t1")
                    nc.vector.scalar_tensor_tensor(
                        t1[:], th[:, 0:3, :], 1.0, th[:, 9:12, :],
                        mybir.AluOpType.add, mybir.AluOpType.mult)
                    t2 = s1pool.tile([128, 3, 32], F32, tag="t2")
                    nc.vector.scalar_tensor_tensor(
                        t2[:], th[:, 3:6, :], 1.0, cring[:, :, j, :].bitcast(F32),
                        mybir.AluOpType.add, mybir.AluOpType.mult)
                    csum = s1pool.tile([128, 3, 32], F32, tag="csum")
                    nc.vector.tensor_add(csum[:], t1[:], t2[:])
                    nc.vector.tensor_scalar_mul(cring[:, :, j + 1, :], csum[:], 0.5)
                    tc_t = s1pool.tile([128, 3, 32], F32, tag="tc")
                    nc.scalar.activation(tc_t[:], csum[:], AF.Tanh, scale=0.5)
                    h_new = hpool.tile([128, 3, 32], BF16, tag="h", name="h_new")
                    nc.vector.scalar_tensor_tensor(
                        h_new[:], th[:, 6:9, :], 1.0, tc_t[:],
                        mybir.AluOpType.add, mybir.AluOpType.mult)
                    h_cur = h_new
                cring_prev = cring
                if DBG:
                    nc.sync.dma_start(c_dbg.ap()[w], cring[:].bitcast(F32))
                    nc.sync.dma_start(h_dbg.ap()[w], h_cur[:])

                # =========== ph2a(w): time-parallel attention ===========
                if PHASES < 2:
                    continue
                def cstar(q):
                    # q 0..2: prev c (t0-1..t0+6); q 3..5: new c (t0..t0+7)
                    if q < 3:
                        return cring[:, q, 0:CH, :]
                    return cring[:, q - 3, 1:CH + 1, :]

                # att1 layer 1 + relu
                y1p = ppb.tile([128, 2, NB], F32, tag="big", name="y1p")
                for mc in range(2):
                    for kc in range(6):
                        nc.tensor.matmul(
                            y1p[:, mc, :], a1w1_t[:, kc, mc * 128:(mc + 1) * 128],
                            cstar(kc), start=(kc == 0), stop=(kc == 5))
                y1 = apool.tile([128, 2, NB], BF16, tag="y1")
                for mc in range(2):
                    nc.scalar.activation(y1[:, mc, :], y1p[:, mc, :], AF.Relu,
                                         bias=a1b1_t[:, mc:mc + 1])
                # att1 layer 2 + exp
                et = apool.tile([128, 6, CH, 32], BF16, tag="et")
                for qp in range(3):
                    ep = ppb.tile([128, 2, NB], F32, tag="big", name="ep")
                    for i in range(2):
                        q = qp * 2 + i
                        for kc in range(2):
                            nc.tensor.matmul(
                                ep[:, i, :], a1w2_t[:, kc, q * 128:(q + 1) * 128],
                                y1[:, kc, :],
                                start=(kc == 0), stop=(kc == 1))
                        nc.scalar.activation(
                            et[:, q, :, :].rearrange("p t b -> p (t b)"),
                            ep[:, i, :], AF.Exp, bias=a1b2_t[:, q:q + 1])
                # S = colsum(E), srow/sinv, broadcast
                ssb = ppb.tile([128, 2, NB], F32, tag="big", name="ssb")
                for q in range(6):
                    nc.tensor.matmul(
                        ssb[0:1, 1, :], ones128_t[:],
                        et[:, q, :, :].rearrange("p t b -> p (t b)"),
                        start=(q == 0), stop=(q == 5))
                srow = apool.tile([1, NB], BF16, tag="srow")
                sinv = apool.tile([1, NB], BF16, tag="sinv")
                with nc.allow_low_precision(reason="f32r rounding of softmax sum is benign"):
                    nc.vector.tensor_copy(srow[:], ssb[0:1, 1, :])
                    nc.vector.reciprocal(sinv[:], ssb[0:1, 1, :])
                nc.tensor.matmul(ssb[:, 0, :], onesr_t[:], sinv[:],
                                 start=True, stop=True)
                sb = apool.tile([128, NB], F32, tag="sb")
                nc.scalar.activation(sb[:], ssb[:, 0, :], AF.Identity)

                # U = E * cStar (unnormalized)
                ut = apool.tile([128, 6, CH, 32], BF16, tag="ut")
                nc.vector.tensor_mul(ut[:, 0:3, :, :], et[:, 0:3, :, :],
                                     cring[:, :, 0:CH, :].bitcast(F32))
                nc.vector.tensor_mul(ut[:, 3:6, :, :], et[:, 3:6, :, :],
                                     cring[:, :, 1:CH + 1, :].bitcast(F32))

                sb3 = sb[:].unsqueeze(1).broadcast_to([128, 2, NB])

                # att2 layer 1: z = relu((a2w1@U + b1*S) * sinv)
                zp = ppb.tile([128, 2, NB], F32, tag="big", name="zp")
                for mc in range(2):
                    nc.tensor.matmul(zp[:, mc, :], a2b1r_t[:, mc * 128:(mc + 1) * 128],
                                     srow[:], start=True, stop=False)
                    for kc in range(6):
                        nc.tensor.matmul(
                            zp[:, mc, :], a2w1_t[:, kc, mc * 128:(mc + 1) * 128],
                            ut[:, kc, :, :].rearrange("p t b -> p (t b)"),
                            start=False, stop=(kc == 5))
                zn = apool.tile([128, 2, NB], F32, tag="zn")
                nc.vector.tensor_mul(zn[:], zp[:], sb3)
                z = apool.tile([128, 2, NB], BF16, tag="z")
                nc.scalar.activation(z[:], zn[:], AF.Relu)

                # att2 layer 2 + tanh -> cHat ring
                chring = chpool.tile([128, 2, CH, 32], F32, tag="ch", name="chring")
                ap2 = ppb.tile([128, 2, NB], F32, tag="big", name="ap2")
                for mc in range(2):
                    for kc in range(2):
                        nc.tensor.matmul(
                            ap2[:, mc, :], a2w2_t[:, kc, mc * 128:(mc + 1) * 128],
                            z[:, kc, :], start=(kc == 0), stop=(kc == 1))
                    nc.scalar.activation(
                        chring[:, mc, :, :].rearrange("p t b -> p (t b)"),
                        ap2[:, mc, :], AF.Tanh, bias=a2b2c_t[:, mc:mc + 1])

                # g1/g2 attended part, normalized: u = (ga@U + b1*S) * sinv
                uring = upool.tile([128, 4, CH, 32], F32, tag="u", name="uring")
                for gi, (gw, gbr) in enumerate(((g1a_t, g1b1r_t), (g2a_t, g2b1r_t))):
                    gp = ppb.tile([128, 2, NB], F32, tag="big", name="gp")
                    for mc in range(2):
                        nc.tensor.matmul(gp[:, mc, :], gbr[:, mc * 128:(mc + 1) * 128],
                                         srow[:], start=True, stop=False)
                        for kc in range(6):
                            nc.tensor.matmul(
                                gp[:, mc, :], gw[:, kc, mc * 128:(mc + 1) * 128],
                                ut[:, kc, :, :].rearrange("p t b -> p (t b)"),
                                start=False, stop=(kc == 5))
                    nc.vector.tensor_mul(
                        uring[:, 2 * gi:2 * gi + 2, :, :].rearrange("p a t b -> p a (t b)"),
                        gp[:], sb3)

                if DBG:
                    nc.sync.dma_start(u_dbg.ap()[w], uring[:])
                    nc.sync.dma_start(ch_dbg.ap()[w], chring[:])
                    nc.sync.dma_start(et_dbg.ap()[w], et[:])
                # =========== ph3(w): memory recurrence ===========
                if PHASES < 3:
                    continue
                for j in range(CH):
                    p3 = pp3.tile([128, 8, 32], F32, tag="p3", name="p3")
                    # qg bias preload (independent of hh)
                    for r in range(4):
                        nc.tensor.matmul(p3[:, 4 + r, :],
                                         gb2r_t[:, r * 128:(r + 1) * 128],
                                         onesb_t[:], start=True, stop=False)
                    # pg = Gmem @ mem
                    for r, gwt in enumerate((g1b_t, g1b_t, g2b_t, g2b_t)):
                        mc = r % 2
                        for kc in range(2):
                            nc.tensor.matmul(
                                p3[:, r, :], gwt[:, kc, mc * 128:(mc + 1) * 128],
                                mem_cur[:, kc, :],
                                start=(kc == 0), stop=(kc == 1))
                    wv = s3pool.tile([128, 4, 32], F32, tag="wv")
                    nc.vector.tensor_add(wv[:], p3[:, 0:4, :], uring[:, :, j, :])
                    hh = s3pool.tile([128, 4, 32], BF16, tag="hh")
                    nc.vector.tensor_scalar_max(hh[:], wv[:], 0.0)
                    # qg += W2 @ hh
                    for r, gwt in enumerate((g1w2_t, g1w2_t, g2w2_t, g2w2_t)):
                        mc = r % 2
                        goff = 0 if r < 2 else 2
                        for kc in range(2):
                            nc.tensor.matmul(
                                p3[:, 4 + r, :], gwt[:, kc, mc * 128:(mc + 1) * 128],
                                hh[:, goff + kc, :],
                                start=False, stop=(kc == 1))
                    th3 = s3pool.tile([128, 4, 32], F32, tag="th3")
                    nc.scalar.activation(th3[:], p3[:, 4:8, :], AF.Tanh)
                    m1 = s3pool.tile([128, 2, 32], F32, tag="m1")
                    nc.vector.scalar_tensor_tensor(
                        m1[:], th3[:, 0:2, :], 1.0, mem_cur[:].bitcast(F32),
                        mybir.AluOpType.add, mybir.AluOpType.mult)
                    m2 = s3pool.tile([128, 2, 32], F32, tag="m2")
                    nc.vector.scalar_tensor_tensor(
                        m2[:], th3[:, 2:4, :], 1.0, chring[:, :, j, :],
                        mybir.AluOpType.add, mybir.AluOpType.mult)
                    msum = s3pool.tile([128, 2, 32], F32, tag="msum")
                    nc.vector.tensor_add(msum[:], m1[:], m2[:])
                    mem_new = mpool.tile([128, 2, 32], F32R, tag="mem", name="mem_new")
                    nc.vector.tensor_scalar_mul(mem_new[:], msum[:], 0.5)
                    mem_cur = mem_new

                if DBG:
                    nc.sync.dma_start(mem_dbg.ap()[w], mem_cur[:].bitcast(F32))

            # =========== ph4: output MLP ===========
            o1p = ppb.tile([128, 2, 32], F32, tag="big", name="o1p")
            rhs5 = [h_cur[:, 0, :], h_cur[:, 1, :], h_cur[:, 2, :],
                    mem_cur[:, 0, :], mem_cur[:, 1, :]]
            for mc in range(2):
                for kc in range(3):
                    nc.tensor.matmul(
                        o1p[:, mc, :], ow1h_t[:, kc, mc * 128:(mc + 1) * 128],
                        rhs5[kc], start=(kc == 0), stop=False)
                for kc in range(2):
                    nc.tensor.matmul(
                        o1p[:, mc, :], ow1m_t[:, kc, mc * 128:(mc + 1) * 128],
                        rhs5[3 + kc], start=False, stop=(kc == 1))
            o1s = s3pool.tile([128, 2, 32], F32R, tag="o1s")
            for mc in range(2):
                nc.scalar.activation(o1s[:, mc, :], o1p[:, mc, :], AF.Relu,
                                     bias=ob1_t[:, mc:mc + 1])
            o2p = ppb.tile([1, 32], F32, tag="big", name="o2p")
            for kc in range(2):
                nc.tensor.matmul(o2p[:], ow2_t[:, kc:kc + 1], o1s[:, kc, :],
                                 start=(kc == 0), stop=(kc == 1))
            o2s = s3pool.tile([1, 32], F32, tag="o2s")
            nc.scalar.activation(o2s[:], o2p[:], AF.Identity)
            nc.sync.dma_start(out_d.ap().rearrange("b one -> (one) (b)"), o2s[:])

    nc.compile()
    return nc


# ---------------------------------------------------------------------------
# host-side packing
# ---------------------------------------------------------------------------

def _bf(x):
    return np.ascontiguousarray(np.asarray(x, np.float32))


def _tobf(x):
    return np.ascontiguousarray(np.asarray(x, np.float32).astype(ml_dtypes.bfloat16))


def pack_shared(inp):
    """Pack weight tensors (identical across cores)."""
    f = np.float32
    d = {}
    wih = {0: inp["Wih_l"], 1: inp["Wih_a"], 2: inp["Wih_v"]}
    whh = {0: inp["Whh_l"], 1: inp["Whh_a"], 2: inp["Whh_v"]}
    bb = {m: (np.asarray(inp[f"bih_{k}"], f) + np.asarray(inp[f"bhh_{k}"], f))
          for m, k in ((0, "l"), (1, "a"), (2, "v"))}
    foff = {0: 0, 1: D_L, 2: D_L + D_A}

    wih5 = np.zeros((5 * 128, 1536), f)
    whhT = np.zeros((128, 1536), f)
    bb12 = np.zeros((128, 12), f)
    for gq in range(4):
        tg = TORCH_G[gq]
        for m in range(3):
            s = gq * 3 + m
            wblk = np.asarray(wih[m], f)[tg * 128:(tg + 1) * 128, :]  # [128, din]
            whhT[:, s * 128:(s + 1) * 128] = np.asarray(whh[m], f)[tg * 128:(tg + 1) * 128, :].T
            bb12[:, s] = bb[m][tg * 128:(tg + 1) * 128]
            # scatter wblk.T rows (x-features of modality m) into the 5-chunk layout
            for kc, (r0, rows, km) in enumerate(XCHUNKS):
                if km != m:
                    continue
                loc0 = r0 - foff[m]  # feature offset within modality
                wih5[kc * 128:kc * 128 + rows, s * 128:(s + 1) * 128] = \
                    wblk[:, loc0:loc0 + rows].T
    # sigmoid-as-tanh folding: sigma(x) = 0.5*(tanh(x/2)+1).
    # sigma-slots (0..8): pre-activation scaled by 0.5 (fold into weights+bias).
    # h is stored doubled (h2 = 2h): whh and out_W1 h-columns scaled by 0.5.
    wih5[:, :9 * 128] *= 0.5
    bb12[:, :9] *= 0.5
    whhT *= 0.5          # h2 compensation (all slots)
    whhT[:, :9 * 128] *= 0.5
    d["wih5"] = _tobf(wih5)
    d["whhT"] = _tobf(whhT)
    d["bb12"] = bb12
    d["ident"] = _bf(np.eye(128, dtype=f))
    d["ones128"] = _tobf(np.ones((128, 1), f))

    d["a1w1"] = _bf(np.asarray(inp["att1_W1"], f).T)              # [768, 256]
    d["a1b1"] = np.asarray(inp["att1_b1"], f).reshape(2, 128).T.copy()
    d["a1w2"] = _tobf(np.asarray(inp["att1_W2"], f).T)              # [256, 768]
    d["a1b2"] = np.asarray(inp["att1_b2"], f).reshape(6, 128).T.copy()
    d["a2w1"] = _tobf(np.asarray(inp["att2_W1"], f).T)
    d["a2b1r"] = _tobf(np.asarray(inp["att2_b1"], f).reshape(1, 256))
    d["a2w2"] = _tobf(np.asarray(inp["att2_W2"], f).T)              # [256, 256]
    d["a2b2c"] = np.asarray(inp["att2_b2"], f).reshape(2, 128).T.copy()
    d["g1a"] = _tobf(np.asarray(inp["g1_W1"], f)[:, :768].T)
    d["g2a"] = _tobf(np.asarray(inp["g2_W1"], f)[:, :768].T)
    d["g1b"] = _bf(np.asarray(inp["g1_W1"], f)[:, 768:].T)
    d["g2b"] = _bf(np.asarray(inp["g2_W1"], f)[:, 768:].T)
    d["g1b1r"] = _tobf(np.asarray(inp["g1_b1"], f).reshape(1, 256))
    d["g2b1r"] = _tobf(np.asarray(inp["g2_b1"], f).reshape(1, 256))
    d["g1w2"] = _tobf(np.asarray(inp["g1_W2"], f).T * 0.5)
    d["g2w2"] = _tobf(np.asarray(inp["g2_W2"], f).T * 0.5)
    d["gb2r"] = _tobf(np.concatenate([np.asarray(inp["g1_b2"], f),
                                    np.asarray(inp["g2_b2"], f)]).reshape(1, 512) * 0.5)
    ow1T = np.asarray(inp["out_W1"], f).T
    d["ow1h"] = _tobf(ow1T[:384] * 0.5)
    d["ow1m"] = _bf(ow1T[384:])
    d["ob1"] = np.asarray(inp["out_b1"], f).reshape(2, 128).T.copy()
    d["ow2"] = _bf(np.asarray(inp["out_W2"], f).reshape(2, 128).T)  # [128, 2]
    return d


def pack_x(x, core, Tp):
    """x: [Tp, 256, 409] -> xT [409, Tp*B] for one core."""
    xc = np.asarray(x[:, core * B:(core + 1) * B, :], np.float32)   # [Tp, B, 409]
    return np.ascontiguousarray(xc.transpose(2, 0, 1).reshape(DIN, Tp * B).astype(ml_dtypes.bfloat16))


_CACHE = {}


def _get_program(Tp):
    if Tp not in _CACHE:
        _CACHE[Tp] = build_program(Tp)
    return _CACHE[Tp]


def kernel(**inputs):
    x = np.asarray(inputs["x"])
    Tp = x.shape[0]
    nc = _get_program(Tp)
    shared = pack_shared(inputs)
    in_maps = []
    for c in range(NCORES):
        m = dict(shared)
        m["xT"] = pack_x(x, c, Tp)
        in_maps.append(m)
    res = run_bass_kernel_spmd(nc, in_maps, list(range(NCORES))).results
    out = np.concatenate([r["out"] for r in res], axis=0)
    out = out + np.asarray(inputs["out_b2"], np.float32).reshape(1, 1)
    return out.astype(np.float32)


if __name__ == "__main__":
    import time
    t0 = time.time()
    nc = build_program(32)
    print("built in", time.time() - t0, "s")
